# revision 26
# baseline (speedup 1.0000x reference)
# CondConv2d Trainium2 kernel, v2 (bf16 + warmup + pipelined prologue).
#
# Math (per sample n=(b,l)):
#   pooled[c]   = mean_{h,w} x[n,c,h,w]
#   allxet      = [p0,p0,p0,p1,p2,p3] temporal window (first frame dup'd twice)
#   calib[c,t]  = conv1d(allxet, tconv_w)[c,t] + tconv_b[c]
#   gate[t]     = conv1d(allxet, fc_w)[0,t] + fc_b
#   scale[n,c]  = calib[c,l] + 1
#   out[n,o]    = conv2d(x[n] * scale[n,:,None,None], weight) + bias[o]*(gate[l]+1)
# (the per-sample weight scale is folded into the input; conv is linear per
#  input channel)
#
# Sharding: data-parallel over b: 8 cores x 2 samples. Weights replicated.
# Conv as implicit GEMM in bf16 (tolerance 2e-2; bf16 end-to-end measured
# ~3e-3): contraction over ci (2 chunks of 128 partitions), 9 shifted-window
# matmuls accumulate in fp32 PSUM.
#
# v2 changes vs baseline (193us):
#  - all matmul operands bf16 (FWL weight loads, half the DMA bytes)
#  - x shipped pre-padded [H, W+2] so tiles DMA contiguously and are scaled
#    in place by DVE (no fp32 staging copy, no pad-col memsets)
#  - 14 dummy warmup matmuls at t=0 keep the PE HAM-warm before the stream
#  - x(b=0) DMA'd before weights; calib split so sample (0,0) only needs
#    frame 0 -> first conv matmul ~6us instead of 26us
#  - b=1 calib emitted between early conv groups (avoids the mid-kernel
#    PE-idle HAM re-throttle)
#  - output shipped bf16, host casts to fp32

import numpy as np
import ml_dtypes


def _install_axon_ntff_shim():
    # This container's `antenv` stub lacks `axon_hooks`, which
    # bass_utils imports unconditionally when trace=True under axon.
    # Provide it (and register the ctypes NTFF hook if the .so is
    # present) so tracing works; missing pieces degrade to no-trace.
    import os
    import sys
    import types

    try:
        import antenv.axon_hooks  # noqa: F401

        return
    except Exception:
        pass
    try:
        import antenv
    except Exception:
        return
    mod = types.ModuleType("antenv.axon_hooks")
    mod._hook = None

    def set_axon_ntff_profile_hook(h):
        mod._hook = h

    def get_axon_ntff_profile_hook():
        return mod._hook

    mod.set_axon_ntff_profile_hook = set_axon_ntff_profile_hook
    mod.get_axon_ntff_profile_hook = get_axon_ntff_profile_hook
    sys.modules["antenv.axon_hooks"] = mod
    antenv.axon_hooks = mod
    try:
        from trn_agent_boot.trn_boot import _ntff_profile_via_ctypes

        so = "/opt/axon/libaxon_pjrt.so"
        if os.path.exists(so):
            mod._hook = _ntff_profile_via_ctypes(so)
    except Exception:
        pass


_install_axon_ntff_shim()

import concourse.bass as bass
import concourse.tile as tile
from concourse import mybir
from concourse.bass_utils import run_bass_kernel_spmd

B, L, CIN, COUT, KS, H, W = 16, 4, 256, 256, 3, 32, 32
NCORES = 8
BS = B // NCORES      # batch samples per core
CC = CIN // 128       # ci chunks
OC = COUT // 128      # co chunks
WP = W + 2            # x tile row width incl. zero pad cols
FP32 = mybir.dt.float32
BF16 = mybir.dt.bfloat16
HHALF = 16            # psum bank = 512 fp32 = 16 rows of 32
NWARM = 16            # dummy matmuls to warm the PE HAM before the stream

_last_results = None  # test harness reads exec_time_ns from here


def _split_excess_waits(nc):
    # walrus in this toolchain encodes exactly one sem wait per engine
    # instruction (TPB_EVENTS has a single wait slot) and optimize_sems
    # is disabled, so Tile can emit instructions with >1 wait that fail
    # codegen ("Too many sync wait commands").  Split the excess waits
    # into standalone EventSemaphore instructions on the same engine
    # stream immediately before the instruction; in-order issue makes
    # this equivalent.  Applies to Drain too (CTRL struct: one wait).
    n = 0
    f = nc.m.functions[0]
    for bb in f.blocks:
        insts = list(bb.instructions)
        out = []
        changed = False
        for inst in insts:
            si = inst.sync_info
            if si is not None:
                waits = list(si.on_wait)
                if len(waits) > 1:
                    for w in waits[:-1]:
                        n += 1
                        es = mybir.InstEventSemaphore(name=f"ES-SPLIT-{n}")
                        es.engine = inst.engine
                        es.sync_info = mybir.SyncInfo(on_wait=[w], on_update=[])
                        out.append(es)
                    si.on_wait = [waits[-1]]
                    inst.sync_info = si
                    changed = True
            out.append(inst)
        if changed:
            bb.instructions = out
    return n


def _strip_mm_incs(nc):
    # Every matmul carries a sem-inc that fires @complete; the EVT_SEM
    # register write serializes with the PE completion pipeline (~26ns
    # per inc).  Consumers only ever need the inc of the LAST matmul of
    # each accumulation group (PE completes in order), so drop the incs
    # from mid-group matmuls and rewrite every wait threshold on the
    # affected semaphores to count only the kept incs.
    f = nc.m.functions[0]
    insts = [i for bb in f.blocks for i in bb.instructions]

    # sems updated exclusively by PE matmuls, all by sem-inc(1)
    upd_src = {}
    for inst in insts:
        si = inst.sync_info
        if not si:
            continue
        for u in si.on_update:
            key = (str(u.sync_type), u.id)
            ok = (
                isinstance(inst, mybir.InstMatmult)
                and str(inst.engine) == "EngineType.PE"
                and u.update_mode == "sem-inc"
                and (u.update_value or 1) == 1
            )
            prev = upd_src.get(key, True)
            upd_src[key] = prev and ok
    target_sems = {k for k, v in upd_src.items() if v}
    if not target_sems:
        return 0

    # per-sem PE-program-order update list: (inst, kept?)
    orders = {k: [] for k in target_sems}
    for inst in insts:
        si = inst.sync_info
        if not si:
            continue
        for u in si.on_update:
            key = (str(u.sync_type), u.id)
            if key in target_sems:
                kept = bool(inst.stop_tensor_calc)
                orders[key].append((inst, u, kept))

    # kept-prefix counts: kp[v] = kept updates among first v
    kp = {}
    removed_at = {}
    for key, lst in orders.items():
        pref = [0]
        rem = [False]
        c = 0
        for _, _, kept in lst:
            c += kept
            pref.append(c)
            rem.append(not kept)
        kp[key] = pref
        removed_at[key] = rem

    # rewrite waits
    n_rewritten = 0
    for inst in insts:
        si = inst.sync_info
        if not si:
            continue
        for w in si.on_wait:
            key = (str(w.sync_type), w.id)
            if key not in target_sems:
                continue
            v = w.wait_value
            if v is None:
                continue
            assert w.wait_mode == "sem-ge-imm", w.wait_mode
            pref = kp[key]
            v = min(v, len(pref) - 1)
            nv = pref[v]
            if removed_at[key][v]:
                # old threshold landed mid-group: wait for the group end
                # instead (later => still safe)
                nv = min(nv + 1, pref[-1])
            w.wait_value = nv
            n_rewritten += 1

    # drop the mid-group incs
    n_removed = 0
    for key, lst in orders.items():
        for inst, u, kept in lst:
            if not kept:
                inst.sync_info.on_update = [
                    x for x in inst.sync_info.on_update if x is not u
                ]
                n_removed += 1
    return n_removed


def build_nc():
    nc = bass.Bass()
    x_d = nc.dram_tensor("x", [BS, L, CIN, H, WP], BF16, kind="ExternalInput")
    w_d = nc.dram_tensor("w", [128, OC, CC, 9, 128], BF16, kind="ExternalInput")
    tcw_d = nc.dram_tensor("tconv", [128, CC, 3, CIN], BF16, kind="ExternalInput")
    fcw_d = nc.dram_tensor("fc", [128, CC, 3], BF16, kind="ExternalInput")
    pf_d = nc.dram_tensor("pf32", [128, CC + OC + 1], FP32, kind="ExternalInput")
    out_d = nc.dram_tensor("out", [BS, L, COUT, H, W], BF16, kind="ExternalOutput")

    with tile.TileContext(nc) as tc:
        with (
            tc.tile_pool(name="singles", bufs=1) as singles,
            tc.tile_pool(name="outp", bufs=6) as outp,
            tc.tile_pool(name="pp_conv", bufs=2, space="PSUM") as pp_conv,
            tc.tile_pool(name="pp_c", bufs=2, space="PSUM") as pp_c,
            tc.tile_pool(name="pp_gb", bufs=1, space="PSUM") as pp_gb,
            tc.tile_pool(name="pp_warm", bufs=1, space="PSUM") as pp_warm,
        ):
            # ---- PE warmup: dependency-light dummy matmuls from t=0 ----
            dummy = singles.tile([128, 640], BF16, tag="dummy")
            nc.vector.memset(dummy[:], 0.0)
            pw = pp_warm.tile([128, 512], FP32, tag="pw")
            for i in range(NWARM):
                nc.tensor.matmul(
                    pw[:, :], lhsT=dummy[:, 0:128], rhs=dummy[:, 128:640],
                    start=True, stop=True,
                )


            # ---- persistent state ----
            allxet = singles.tile([128, CC, BS, L + 2], BF16, tag="allxet")
            s_sb = singles.tile([128, CC, BS, L], FP32, tag="s")
            g_sb = singles.tile([1, BS, L], BF16, tag="g")
            fb_sb = singles.tile([128, BS, L, OC], FP32, tag="fb")
            ones_sb = singles.tile([1, 128], BF16, tag="ones")
            nc.vector.memset(ones_sb[:], 1.0)

            w_sb = singles.tile([128, OC, CC, 9, 128], BF16, tag="w")
            tcw_sb = singles.tile([128, CC, 3, CIN], BF16, tag="tcw")
            fcw_sb = singles.tile([128, CC, 3], BF16, tag="fcw")
            pf_sb = singles.tile([128, CC + OC + 1], FP32, tag="pf")

            x_t = {}
            for b in range(BS):
                for l in range(L):
                    for ci in range(CC):
                        x_t[(b, l, ci)] = singles.tile(
                            [128, H, WP], BF16, tag=f"x{b}_{l}_{ci}",
                            name=f"x{b}_{l}_{ci}",
                        )

            def load_pool(b, l, ci):
                xt = x_t[(b, l, ci)]
                nc.sync.dma_start(
                    out=xt[:], in_=x_d[b, l, ci * 128:(ci + 1) * 128, :, :]
                )
                # pad cols are zero (host-padded): summing the whole tile
                # equals the H*W spatial sum; 1/(H*W) is folded into the
                # conv1d weights on host
                # bf16 pooled sums: rel err ~0.4% on a value that only
                # perturbs the conv scale by ~5%, i.e. ~2e-4 end-to-end
                with nc.allow_low_precision(reason="bf16 pool, tol 2e-2"):
                    nc.vector.reduce_sum(
                        out=allxet[:, ci, b, 2 + l:3 + l],
                        in_=xt[:],
                        axis=mybir.AxisListType.XY,
                    )

            def dup_first(b):
                for ci in range(CC):
                    nc.vector.tensor_copy(allxet[:, ci, b, 0:1], allxet[:, ci, b, 2:3])
                    nc.vector.tensor_copy(allxet[:, ci, b, 1:2], allxet[:, ci, b, 2:3])

            def calib_scale(b, t0, t1):
                # scale cols [t0, t1) for sample b; window t uses allxet cols
                # [t, t+3) so only pooled frames <= t1-1 are needed
                n = t1 - t0
                for oc in range(OC):
                    pc = pp_c.tile([128, L], FP32, tag="pc")
                    mms = [(ci, k) for ci in range(CC) for k in range(3)]
                    for i, (ci, k) in enumerate(mms):
                        nc.tensor.matmul(
                            pc[:, 0:n],
                            lhsT=tcw_sb[:, ci, k, oc * 128:(oc + 1) * 128],
                            rhs=allxet[:, ci, b, t0 + k:t1 + k],
                            start=(i == 0),
                            stop=(i == len(mms) - 1),
                        )
                    # scale = calib + tconv_b + 1 (tb+1 precomputed on host)
                    nc.vector.tensor_scalar_add(
                        s_sb[:, oc, b, t0:t1], pc[:, 0:n], pf_sb[:, oc:oc + 1]
                    )

            def calib_gate(b):
                # per-sample output bias gate; only needed by the output
                # copies, so it runs after the first conv group is underway
                pg = pp_c.tile([128, L], FP32, tag="pc", name="pg")
                mms = [(ci, k) for ci in range(CC) for k in range(3)]
                for i, (ci, k) in enumerate(mms):
                    nc.tensor.matmul(
                        pg[0:1, 0:L],
                        lhsT=fcw_sb[:, ci, k:k + 1],
                        rhs=allxet[:, ci, b, k:k + L],
                        start=(i == 0),
                        stop=(i == len(mms) - 1),
                    )
                nc.vector.tensor_scalar_add(
                    g_sb[0:1, b, :], pg[0:1, 0:L],
                    pf_sb[0:1, CC + OC:CC + OC + 1],
                )
                # broadcast (gate+fc_b+1) across partitions via rank-1 matmul
                gb = pp_gb.tile([128, L], FP32, tag="gb")
                nc.tensor.matmul(
                    gb[:, 0:L], lhsT=ones_sb[0:1, :], rhs=g_sb[0:1, b, :],
                    start=True, stop=True,
                )
                for l in range(L):
                    for oc in range(OC):
                        nc.vector.tensor_mul(
                            fb_sb[:, b, l, oc:oc + 1],
                            gb[:, l:l + 1],
                            pf_sb[:, CC + oc:CC + oc + 1],
                        )

            def scale(b, l):
                # in-place bf16 scale; pad cols stay zero (0 * s == 0)
                for ci in range(CC):
                    nc.vector.tensor_scalar_mul(
                        x_t[(b, l, ci)][:], x_t[(b, l, ci)][:],
                        s_sb[:, ci, b, l:l + 1],
                    )

            def conv(b, l, oc, split_out=False):
                ps = pp_conv.tile([128, H, W], FP32, tag="convps")
                for half in range(H // HHALF):
                    h0 = half * HHALF
                    group = []
                    for ci in range(CC):
                        for kh in range(3):
                            dh = kh - 1
                            hA = max(h0, -dh)
                            hB = min(h0 + HHALF, H - dh)
                            if hB <= hA:
                                continue
                            for kw in range(3):
                                group.append((ci, kh, kw, hA, hB))
                    for i, (ci, kh, kw, hA, hB) in enumerate(group):
                        dh = kh - 1
                        nc.tensor.matmul(
                            ps[:, hA:hB, :],
                            lhsT=w_sb[:, oc, ci, kh * 3 + kw, :],
                            rhs=x_t[(b, l, ci)][:, hA + dh:hB + dh, kw:kw + W],
                            start=(i == 0),
                            stop=(i == len(group) - 1),
                        )
                if split_out:
                    # quarter-tile copy+DMA: the early copies overlap the
                    # remaining matmuls (half 0 is a different psum bank),
                    # shortening the kernel tail after the final matmul
                    HQ = 8
                    for q in range(H // HQ):
                        h0 = q * HQ
                        osbh = outp.tile([128, HQ, W], BF16, tag="osbh")
                        nc.vector.tensor_scalar_add(
                            osbh[:], ps[:, h0:h0 + HQ, :],
                            fb_sb[:, b, l, oc:oc + 1],
                        )
                        nc.gpsimd.dma_start(
                            out=out_d[b, l, oc * 128:(oc + 1) * 128,
                                      h0:h0 + HQ, :],
                            in_=osbh[:],
                        )
                else:
                    osb = outp.tile([128, H, W], BF16, tag="osb")
                    nc.vector.tensor_scalar_add(
                        osb[:], ps[:], fb_sb[:, b, l, oc:oc + 1]
                    )
                    nc.gpsimd.dma_start(
                        out=out_d[b, l, oc * 128:(oc + 1) * 128, :, :],
                        in_=osb[:],
                    )

            # ---- emission in expected execution order ----
            # keep the early DMA queue light: only what the first conv
            # group's dependency chain needs (frame-0 tiles, tconv for
            # calib, biases, w oc=0).  The SDMA engines round-robin all
            # queued transfers at packet granularity, so a deferred x
            # flood directly delays the tconv/w completion receipts that
            # gate the first conv matmuls.
            load_pool(0, 0, 0)
            load_pool(0, 0, 1)
            # w(oc0,ci0) feeds the first 9 conv matmuls; tcw gates calib.
            # Both receipts are chain-critical, so interleave small pieces
            nc.sync.dma_start(out=w_sb[:, 0, 0], in_=w_d[:, 0, 0])
            nc.sync.dma_start(out=tcw_sb[:], in_=tcw_d[:])
            nc.sync.dma_start(out=pf_sb[:], in_=pf_d[:])
            nc.sync.dma_start(out=w_sb[:, 0, 1], in_=w_d[:, 0, 1])

            dup_first(0)
            calib_scale(0, 0, 1)  # scale col 0: only frame 0 needed
            scale(0, 0)

            for l in range(1, L):
                for ci in range(CC):
                    load_pool(0, l, ci)
            nc.sync.dma_start(out=fcw_sb[:], in_=fcw_d[:])
            nc.sync.dma_start(out=w_sb[:, 1], in_=w_d[:, 1])

            conv(0, 0, 0)

            calib_scale(0, 1, L)  # scale cols 1..3 after frames 1..3 pooled
            calib_gate(0)
            for l in range(1, L):
                scale(0, l)

            conv(0, 0, 1)

            for l in range(L):
                for ci in range(CC):
                    load_pool(1, l, ci)

            conv(0, 1, 0)

            dup_first(1)
            calib_scale(1, 0, L)
            calib_gate(1)
            for l in range(L):
                scale(1, l)

            conv(0, 1, 1)
            for l in range(2, L):
                for oc in range(OC):
                    conv(0, l, oc)
            for l in range(L):
                for oc in range(OC):
                    conv(1, l, oc, split_out=(l == L - 1 and oc == OC - 1))
    import os
    if os.environ.get("STRIP_MM_INCS", "0") == "1":
        _strip_mm_incs(nc)
    _split_excess_waits(nc)
    return nc


def kernel(x, weight, bias, tconv_w, tconv_b, fc_w, fc_b):
    global _last_results
    x = np.asarray(x, dtype=np.float32)
    weight = np.asarray(weight, dtype=np.float32)
    bias = np.asarray(bias, dtype=np.float32)
    tconv_w = np.asarray(tconv_w, dtype=np.float32)
    tconv_b = np.asarray(tconv_b, dtype=np.float32)
    fc_w = np.asarray(fc_w, dtype=np.float32)
    fc_b = np.asarray(fc_b, dtype=np.float32)

    # host-side packing (shared across cores)
    # x: bf16, padded with zero cols left/right of each row
    x_pad = np.zeros((B, L, CIN, H, WP), dtype=ml_dtypes.bfloat16)
    x_pad[:, :, :, :, 1:W + 1] = x.astype(ml_dtypes.bfloat16)
    # w[p, oc, ci, k, m] = weight[oc*128+m, ci*128+p, kh, kw]
    w_host = np.ascontiguousarray(
        weight.reshape(OC, 128, CC, 128, 3, 3)
        .transpose(3, 0, 2, 4, 5, 1)
        .reshape(128, OC, CC, 9, 128)
        .astype(ml_dtypes.bfloat16)
    )
    # 1/(H*W) pooling normalization folded into the conv1d weights
    tcw_host = np.ascontiguousarray(
        (tconv_w / (H * W)).transpose(1, 2, 0)
        .reshape(CC, 128, 3, CIN).transpose(1, 0, 2, 3)
        .astype(ml_dtypes.bfloat16)
    )
    fcw_host = np.ascontiguousarray(
        (fc_w[0] / (H * W)).reshape(CC, 128, 3).transpose(1, 0, 2)
        .astype(ml_dtypes.bfloat16)
    )
    pf_host = np.zeros((128, CC + OC + 1), dtype=np.float32)
    pf_host[:, 0:CC] = (tconv_b + 1.0).reshape(CC, 128).T
    pf_host[:, CC:CC + OC] = bias.reshape(OC, 128).T
    pf_host[:, CC + OC] = fc_b[0] + 1.0

    nc = build_nc()
    in_maps = []
    for core in range(NCORES):
        in_maps.append({
            "x": np.ascontiguousarray(x_pad[core * BS:(core + 1) * BS]),
            "w": w_host,
            "tconv": tcw_host,
            "fc": fcw_host,
            "pf32": pf_host,
        })
    res = run_bass_kernel_spmd(nc, in_maps, core_ids=list(range(NCORES)))
    _last_results = res
    out = np.concatenate(
        [
            r["out"].astype(np.float32).reshape(BS * L, COUT, H, W)
            for r in res.results
        ],
        axis=0,
    )
    return out
